# revision 21
# baseline (speedup 1.0000x reference)
"""Trainium2 Bass kernel for nn_Cholesky_from_z.

Math: the reference's per-column scan has the closed form
    out[b,i,j] = z[b,i,j] * sqrt( prod_{k<j} (1 - z[b,i,k]^2) )   for j < i
    out[b,i,i] = 1,   out[b,i,j>i] = 0
i.e. a per-row exclusive cumulative product over the packed strictly-lower
entries.  On-device: one hardware prefix scan (tensor_tensor_scan, segmented
via a 0/1 mask with state=(mask max state)*t), a Sqrt pass, one shifted
multiply, then packed->dense copies.  Pure data parallel over batch:
512 items -> 64 per NeuronCore across 8 cores.

Engine discipline: walrus allows at most ONE semaphore wait per compute
instruction, and Tile elides repeated cross-engine waits per engine — the
program is arranged so every instruction depends on at most one foreign
engine tick that its issuing engine has not already observed (per-engine ot
pools, DVE-only Z/T producers, block-0 on DVE because rows 1/181 read the
raw input-DMA bytes at packed position 0).
"""

import dataclasses
import sys

import numpy as np

for _p in ("/opt/trn_rl_repo",):
    if _p not in sys.path:
        sys.path.insert(0, _p)

import concourse.bass as bass
import concourse.tile as tile
from concourse import mybir

# ---------------------------------------------------------------- constants
N = 256                      # matrix dim
B = 512                      # total batch
M = N * (N - 1) // 2         # 32640 packed entries
NCORES = 8
BC = B // NCORES             # 64 batch items per core


def off(i):
    return i * (i - 1) // 2


SPLIT = off(181)             # 16290: half A = rows 1..180, half B = rows 181..255
FREE = 16350                 # A reads vec[:, 0:16350), B reads vec[:, 16290:32640)

A_ROWS = list(range(1, 181))     # rows on partitions 0:64, chunk at off(i)
B_ROWS = list(range(181, 256))   # rows on partitions 64:128, chunk at off(i)-SPLIT

F32 = mybir.dt.float32
BF16 = mybir.dt.bfloat16
U32 = mybir.dt.uint32

ZZ_SPAN = 512
SPAN_BOUNDS = [0, 4096, 8192, 12288, FREE]
XSPLIT = 8192                # scan split: DVE does [0,XSPLIT), GpSimd the rest
XROW_A = 128                 # off(128)=8128 <= 8192 < off(129)=8256
XROW_B = 221                 # off(221)-SPLIT=8020 <= 8192 < off(222)-SPLIT=8241

NOT_TILES = 5                # dedicated output staging tiles
PAIRED_K = 9                 # paired out-blocks k=0..8: A rows 8k..8k+7 | B rows 181+8k..188+8k


def _a_foff(i):
    return off(i)


def _b_foff(i):
    return off(i) - SPLIT


def build_mask():
    """(128, FREE) bf16 segment-start mask: 1.0 at each row's first element."""
    import ml_dtypes

    mask = np.zeros((128, FREE), dtype=np.float32)
    for i in A_ROWS:
        mask[0:64, _a_foff(i)] = 1.0
    for i in B_ROWS:
        mask[64:128, _b_foff(i)] = 1.0
    return mask.astype(ml_dtypes.bfloat16)


def _fix_positions():
    fa = [_a_foff(i) - 1 for i in A_ROWS if _a_foff(i) - 1 >= 0]
    fb = [_b_foff(i) - 1 for i in B_ROWS if _b_foff(i) - 1 >= 0]
    # span-chain reads T[:, f0-1] — assert no fix position sits there
    for f0 in SPAN_BOUNDS[1:-1]:
        assert (f0 - 1) not in fa and (f0 - 1) not in fb, f0
    return fa, fb


def build_nc():
    nc = bass.Bass()
    vec_in = nc.declare_dram_parameter("vec", [BC, M], F32, isOutput=False)
    mask_in = nc.declare_dram_parameter("mask", [128, FREE], BF16, isOutput=False)
    out_d = nc.declare_dram_parameter("out", [BC, N, N], F32, isOutput=True)

    fa, fb = _fix_positions()
    spans = list(zip(SPAN_BOUNDS[:-1], SPAN_BOUNDS[1:]))
    ns = len(spans)

    with tile.TileContext(nc) as tc:
        with (
            tc.tile_pool(name="zp", bufs=1) as zp,
            tc.tile_pool(name="tp", bufs=1) as tp,
            tc.tile_pool(name="mp", bufs=1) as mp,
            tc.tile_pool(name="opd", bufs=1) as opd,
        ):
            # per-span tiles: whole-tile dependency tracking would otherwise
            # serialize every op touching a shared Z/T
            Zs, Ts = [], []
            for si, (f0, f1) in enumerate(spans):
                ln = f1 - f0
                Zs.append(zp.tile([128, ln + 2], F32, tag=f"z{si}", name=f"Zt{si}"))
                # T_s col 0 = chain-in (CP[f0-1]); cols 1..ln = span data
                Ts.append(tp.tile([128, ln + 1], F32, tag=f"t{si}", name=f"Tt{si}"))
            MK = mp.tile([128, FREE + 2], BF16, name="MK")

            nc.vector.memset(Zs[-1][:, spans[-1][1] - spans[-1][0] :], 0.0)

            # ---- input spans, one DMA per partition half (descriptor spread) ----
            for si, (f0, f1) in enumerate(spans):
                for h in range(2):
                    src = dataclasses.replace(
                        vec_in[:, :],
                        ap=[[M, 64], [1, f1 - f0]],
                        offset=h * SPLIT + f0,
                    )
                    nc.sync.dma_start(
                        out=Zs[si][64 * h : 64 * h + 64, 0 : f1 - f0], in_=src
                    )

            nc.sync.dma_start(out=MK[:, 0:FREE], in_=mask_in[:, :])
            nc.gpsimd.memset(MK[:, FREE : FREE + 2], 0.0)

            # ---- per span: square (DVE) -> sqrt(1-x) (ACT, in place) ----
            for si, (f0, f1) in enumerate(spans):
                ln = f1 - f0
                nc.vector.tensor_mul(
                    Ts[si][:, 1 : 1 + ln], Zs[si][:, 0:ln], Zs[si][:, 0:ln]
                )
                nc.scalar.activation(
                    Ts[si][:, 1 : 1 + ln], Ts[si][:, 1 : 1 + ln],
                    mybir.ActivationFunctionType.Sqrt,
                    bias=1.0, scale=-1.0,
                )

            # chain-in for span 0: exactly 1.0 (MK[:,0] is a row start)
            nc.vector.tensor_copy(Ts[0][:, 0:1], MK[:, 0:1])

            # ---- chained segmented cumprod scans (DVE) ----
            for si, (f0, f1) in enumerate(spans):
                ln = f1 - f0
                nc.vector.tensor_tensor_scan(
                    Ts[si][:, 1 : 1 + ln],
                    MK[:, f0:f1],
                    Ts[si][:, 1 : 1 + ln],
                    Ts[si][:, 0:1],
                    op0=mybir.AluOpType.max,
                    op1=mybir.AluOpType.mult,
                )
                if si + 1 < ns:
                    nc.vector.tensor_copy(Ts[si + 1][:, 0:1], Ts[si][:, ln : ln + 1])

            # ---- boundary fix: CP[off_i - 1] := 1.0, done as
            #      T = max(T, MK shifted left by one) since CP <= 1 ----
            for si, (f0, f1) in enumerate(spans):
                ln = f1 - f0
                eng = nc.vector
                eng.tensor_tensor(
                    Ts[si][:, 1 : 1 + ln],
                    Ts[si][:, 1 : 1 + ln],
                    MK[:, f0 + 1 : f1 + 1],
                    mybir.AluOpType.max,
                )

            # ---- shifted multiply: Z[j] *= CP[j-1] (T_s col k = CP[f0+k-1]) ----
            for si, (f0, f1) in enumerate(spans):
                ln = f1 - f0
                nc.vector.tensor_mul(
                    Zs[si][:, 0:ln], Zs[si][:, 0:ln], Ts[si][:, 0:ln]
                )

            # ---- output blocks: 5 dedicated staging tiles, rotated in
            #      increasing-row-length order (zero fill once per tile) ----
            ot_tiles = [opd.tile([128, 8 * N], F32, tag=f"ot{j}", name=f"otile{j}")
                        for j in range(NOT_TILES)]

            def span_of(g):
                for si, (f0, f1) in enumerate(spans):
                    if g < f1:
                        return si
                return ns - 1

            def row_copy(half, i, ot, s):
                ln = i + (i & 1)  # even length -> DVE 2x fp32 copy mode
                fo = _a_foff(i) if half == 0 else _b_foff(i)
                p0 = 64 * half
                dcol = s * N
                g = fo
                while g < fo + ln:
                    si = span_of(g)
                    f0, f1 = spans[si]
                    zlim = f1 if si + 1 < ns else f1 + 2
                    take = min(fo + ln, zlim) - g
                    dst = ot[p0 : p0 + 64, dcol : dcol + take]
                    src_ap = Zs[si][p0 : p0 + 64, g - f0 : g - f0 + take]
                    if take >= 160:
                        nc.vector.tensor_copy(dst, src_ap)
                    else:
                        nc.scalar.copy(dst, src_ap)
                    g += take
                    dcol += take

            def emit_block(rows_a, rows_b, r0a, r0b, ot, first):
                nrow = max(len(rows_a), len(rows_b))
                if first:
                    nc.gpsimd.memset(ot[:, :], 0.0)
                for s, i in enumerate(rows_a):
                    if i == 0:
                        continue
                    row_copy(0, i, ot, s)
                for s, i in enumerate(rows_b):
                    row_copy(1, i, ot, s)
                if rows_a:
                    na = len(rows_a)
                    nc.gpsimd.memset(ot[0:64, r0a : r0a + 257 * (na - 1) + 1 : 257], 1.0)
                if rows_b:
                    nb = len(rows_b)
                    nc.gpsimd.memset(ot[64:128, r0b : r0b + 257 * (nb - 1) + 1 : 257], 1.0)
                return nrow

            def out_dma(ot, half, r0, nrow):
                dst = dataclasses.replace(
                    out_d[:, :, :],
                    ap=[[N * N, 64], [1, nrow * N]],
                    offset=r0 * N,
                )
                p0 = 64 * half
                nc.sync.dma_start(out=dst, in_=ot[p0 : p0 + 64, 0 : nrow * N])

            blocks = []
            for k in range(PAIRED_K):
                blocks.append((list(range(8 * k, 8 * k + 8)),
                               list(range(181 + 8 * k, 189 + 8 * k))))
            a_rest = list(range(72, 181))
            for b0 in range(0, len(a_rest), 8):
                blocks.append((a_rest[b0 : b0 + 8], []))
            blocks.append(([], [253, 254, 255]))

            for j, (ra, rb) in enumerate(blocks):
                ot = ot_tiles[j % NOT_TILES]
                nrow = emit_block(ra, rb, ra[0] if ra else 0, rb[0] if rb else 0,
                                  ot, first=(j < NOT_TILES))
                if ra:
                    out_dma(ot, 0, ra[0], len(ra))
                if rb:
                    out_dma(ot, 1, rb[0], len(rb))

    return nc


def _split_multi_waits(nc):
    """Walrus accepts at most one semaphore wait per engine instruction.
    Tile sometimes emits several — hoist all but the last onto standalone
    same-engine Drain instructions inserted immediately before."""
    cnt = [0]

    def carrier(engine, wait):
        cnt[0] += 1
        d = mybir.InstDrain(name=f"I-waitsplit-{cnt[0]}", ins=[], outs=[])
        d.engine = engine
        d.sync_info = mybir.SyncInfo(on_wait=[wait], on_update=[])
        return d

    for blk in nc.m.functions[0].blocks:
        lst = blk.instructions
        out = []
        for inst in lst:
            si = getattr(inst, "sync_info", None)
            waits = list(si.on_wait) if si is not None else []
            if len(waits) > 1:
                for w in waits[:-1]:
                    out.append(carrier(inst.engine, w))
                inst.sync_info = mybir.SyncInfo(
                    on_wait=[waits[-1]], on_update=list(si.on_update)
                )
            out.append(inst)
        lst[:] = out


_CACHE = {}


def _get_nc():
    if "nc" not in _CACHE:
        nc = build_nc()
        _split_multi_waits(nc)   # HW path only; CoreSim uses raw build_nc()
        _CACHE["nc"] = nc
    return _CACHE["nc"]


TRACE = False


def kernel(vec):
    vec = np.ascontiguousarray(vec, dtype=np.float32)
    assert vec.shape == (B, M), vec.shape
    from concourse.bass_utils import run_bass_kernel_spmd

    nc = _get_nc()
    mask = build_mask()
    in_maps = [
        {"vec": vec[c * BC : (c + 1) * BC], "mask": mask} for c in range(NCORES)
    ]
    res = run_bass_kernel_spmd(nc, in_maps, list(range(NCORES)), trace=TRACE)
    if TRACE:
        _CACHE["last_exec_time_ns"] = res.exec_time_ns
        _CACHE["last_results"] = res
    out = np.concatenate([res.results[c]["out"] for c in range(NCORES)], axis=0)
    return out.astype(np.float32)
